# revision 29
# baseline (speedup 1.0000x reference)
"""Trainium2 Bass kernel for nn_DDPMVAEQueryEncoder.

Strategy (data-parallel over batch, 8 cores):
  * Pooling is split across two independent gather resources per core:
      - chunks 0,1 (leanest bands): int16 `dma_gather` from the fp32 table in
        DRAM (4 range-segments, greedy band packing to minimize padding),
        DVE tree-reduce, PE transpose — as before.
      - chunks 2,3: the fp16 table lives in SBUF packed 2-dims-per-uint32
        ([128, 25001]: partition 32k+q = dims (2q,2q+1) of vocab quarter k);
        gpsimd `ap_gather` fetches all 200 slots/row with zero padding, DVE
        pairwise-tree reduces the packed fp16 pairs, quarters fold by
        partition-halving, rank-1 rsq scale, permuted-wec matmuls.
    The DMA engine and the gpsimd engine run concurrently, halving the
    serialized gather time that dominated the kernel.
  * Chains pair one DMA chunk with one SBUF chunk (h0={c0,c2}, h1={c1,c3})
    so both start as early as possible; 50 ancestral DDPM steps per chain,
    fp32r matmuls; per-step tail: silu acts on ACT, x-update on DVE.
  * Host folds: W_enc/Wc, timestep tables, schedule constants, sigma and
    -1/C into the fp16 noise, 1/sqrt(nnz) per row.
"""
import sys

import numpy as np

if "/opt/trn_rl_repo" not in sys.path:
    sys.path.insert(0, "/opt/trn_rl_repo")

import concourse.bass as bass
import concourse.mybir as mybir
import concourse.tile as tile
from concourse import bacc
from concourse.bass_utils import run_bass_kernel_spmd
from concourse.masks import make_identity

F32 = mybir.dt.float32
F32R = mybir.dt.float32r
F16 = mybir.dt.float16
U32 = mybir.dt.uint32
I16 = mybir.dt.int16

T_STEPS = 50
D = 64
B = 4096
L = 200
V = 100000
NCORES = 8
BL = B // NCORES          # 512 rows per core
HB = BL // 2              # 256 per half-batch chain
NCHUNK = 4
NSEG = 4
SEG = 25000               # index range per dma_gather segment
SEGR = SEG + 1            # +1 zero row
Q = 25000                 # vocab quarter for the SBUF table
DMA_CHUNKS = (0, 1)
SBUF_CHUNKS = (2, 3)
# xc column order: chain h0 = [c0 | c2], chain h1 = [c1 | c3]
CHAIN_CHUNKS = ((0, 2), (1, 3))



def _schedule_consts():
    steps = T_STEPS
    scale = 1000.0 / steps
    betas = np.linspace(scale * 1e-4, scale * 2e-2, steps, dtype=np.float64)
    alphas = 1.0 - betas
    acp = np.cumprod(alphas)
    acp_prev = np.append(1.0, acp[:-1])
    sqrt_recip = np.sqrt(1.0 / acp)
    sqrt_recipm1 = np.sqrt(1.0 / acp - 1.0)
    post_var = betas * (1.0 - acp_prev) / (1.0 - acp)
    post_logvar = np.log(np.append(post_var[1], post_var[1:]))
    coef1 = betas * np.sqrt(acp_prev) / (1.0 - acp)
    coef2 = (1.0 - acp_prev) * np.sqrt(alphas) / (1.0 - acp)
    A = (coef1 * sqrt_recip + coef2).astype(np.float32)
    C = (coef1 * sqrt_recipm1).astype(np.float32)
    S = np.exp(0.5 * post_logvar).astype(np.float32)
    S[0] = 0.0
    return A, C, S


def _timestep_tables(Wt, bt, W1, b1):
    half = D // 2
    freqs = np.exp(-np.log(10000.0) * np.arange(half, dtype=np.float32) / half)
    t = np.arange(T_STEPS, dtype=np.float32)
    args = t[:, None] * freqs[None, :]
    temb = np.concatenate([np.cos(args), np.sin(args)], axis=-1).astype(np.float32)
    tt = (temb @ Wt + bt).astype(np.float32)
    return (b1 + tt @ W1).astype(np.float32)  # [50, 256]


def host_prep(inputs):
    seq = np.asarray(inputs["seq"]).astype(np.int64)
    item_emb = np.asarray(inputs["item_emb"], dtype=np.float32)
    W_enc = np.asarray(inputs["W_enc"], dtype=np.float32)
    b_enc = np.asarray(inputs["b_enc"], dtype=np.float32)
    Wt = np.asarray(inputs["Wt"], dtype=np.float32)
    bt = np.asarray(inputs["bt"], dtype=np.float32)
    Wc = np.asarray(inputs["Wc"], dtype=np.float32)
    bc = np.asarray(inputs["bc"], dtype=np.float32)
    W1 = np.asarray(inputs["W1"], dtype=np.float32)
    b1 = np.asarray(inputs["b1"], dtype=np.float32)
    W2 = np.asarray(inputs["W2"], dtype=np.float32)
    b2 = np.asarray(inputs["b2"], dtype=np.float32)
    init_noise = np.asarray(inputs["init_noise"], dtype=np.float32)
    step_noise = np.asarray(inputs["step_noise"], dtype=np.float32)

    A, C, S = _schedule_consts()

    # row permutation: greedy-pack rows into 4 bands of 1024 minimizing the
    # per-band per-range max counts (= dma_gather padding); leanest bands go
    # to the DMA chunks (0,1), the SBUF ap_gather path (2,3) has no padding.
    bucket = seq // SEG
    counts = np.stack([(bucket == k).sum(1) for k in range(NSEG)], 1)
    mx = counts.max(1)
    idx_desc = np.argsort(-mx, kind="stable")
    bands = [[] for _ in range(NCHUNK)]
    bmax = np.zeros((NCHUNK, NSEG), np.int64)
    for r in idx_desc:
        best, bestcost = None, None
        for b in range(NCHUNK):
            if len(bands[b]) >= NCORES * 128:
                continue
            cost = np.maximum(bmax[b], counts[r]).sum() - bmax[b].sum()
            if bestcost is None or cost < bestcost:
                best, bestcost = b, cost
        bands[best].append(r)
        bmax[best] = np.maximum(bmax[best], counts[r])
    border = np.argsort(bmax.sum(1), kind="stable")    # leanest first
    order = np.concatenate([np.array(bands[b]) for b in border])
    rows = order.reshape(NCHUNK, NCORES, 128)          # [chunk, core, row]

    tbl = np.zeros((NSEG * SEGR, D), np.float32)
    for k in range(NSEG):
        tbl[k * SEGR: k * SEGR + SEG] = item_emb[k * SEG: (k + 1) * SEG]

    G = counts[order].reshape(NCHUNK, NCORES * 128, NSEG).max(1)
    G = np.maximum(G, 1).astype(np.int64)[list(DMA_CHUNKS)]   # [2, 4]

    # int16 gather index tiles for DMA chunks
    idx16 = [[[None] * NSEG for _ in range(2)] for _ in range(NCORES)]
    for ci, c in enumerate(DMA_CHUNKS):
        for n in range(NCORES):
            rs = rows[c, n]
            sq = seq[rs]
            bk = bucket[rs]
            for k in range(NSEG):
                g = int(G[ci, k])
                val = np.full((128, g), SEG, np.int16)
                for p in range(128):
                    e = sq[p][bk[p] == k] - k * SEG
                    val[p, : len(e)] = e.astype(np.int16)
                v = val.reshape(8, 16, g)              # [p//16, p%16, g]
                arr = np.transpose(v, (1, 2, 0)).reshape(16, g * 8)
                idx16[n][ci][k] = np.ascontiguousarray(np.tile(arr, (8, 1)))

    # packed fp16 table [128, Q+1]: partition 32k+q = dims (2q,2q+1), quarter k
    t16 = item_emb.astype(np.float16)
    tbl32 = np.zeros((128, Q + 1), np.uint32)
    for k in range(4):
        sl = t16[k * Q:(k + 1) * Q]                       # [Q, 64]
        pair = sl.view(np.uint16).reshape(Q, D // 2, 2)
        packed = (pair[:, :, 0].astype(np.uint32)
                  | (pair[:, :, 1].astype(np.uint32) << 16))
        tbl32[32 * k:32 * (k + 1), 1:] = packed.T

    # ap_gather idx: ONE call per sbuf-chunk. Per 16-partition group g
    # (quarter k=g//2): per-row padded rectangles of that quarter's hits.
    # GS[ci] = band max count over rows x quarters; pad idx 0 -> zero col.
    GS = counts[order].reshape(NCHUNK, NCORES * 128, NSEG).max((1, 2))
    GS = np.maximum(GS, 1).astype(np.int64)[list(SBUF_CHUNKS)]   # [2]
    apgidx = [[None] * 2 for _ in range(NCORES)]
    for ci, c in enumerate(SBUF_CHUNKS):
        gs = int(GS[ci])
        for n in range(NCORES):
            sq = seq[rows[c, n]]                          # [128, 200]
            bk = bucket[rows[c, n]]
            t = np.zeros((128, 8 * gs), np.int16)
            for k in range(4):
                val = np.zeros((128, gs), np.int16)
                for p in range(128):
                    e = sq[p][bk[p] == k] - k * Q + 1
                    val[p, : len(e)] = e.astype(np.int16)
                # j = r*gs + l -> [j%16, j//16]
                wrapped = val.reshape(-1).reshape(8 * gs, 16).T
                t[32 * k:32 * k + 16] = wrapped
                t[32 * k + 16:32 * k + 32] = wrapped
            apgidx[n][ci] = np.ascontiguousarray(t)

    nnz = np.maximum(np.count_nonzero(seq, axis=1), 1).astype(np.float32)
    rsq_all = (1.0 / np.sqrt(nnz))                        # [B]

    wec = (W_enc[:, :D] @ Wc).astype(np.float32)
    # permuted wec for the packed layout: lhsT_e [32, 64] = wec rows (2q+e)
    wec_p = np.stack([wec[0::2, :], wec[1::2, :]], 0).astype(np.float32)
    bec = (b_enc[:D] @ Wc + bc).astype(np.float32).reshape(D, 1)
    w1s = np.vstack([W1, W1]).astype(np.float32)          # [128, 256]
    TB1 = _timestep_tables(Wt, bt, W1, b1)
    tb1 = np.ascontiguousarray(
        np.concatenate([TB1[:, :128].T, TB1[:, 128:].T], axis=1))  # [128, 100]
    # scaled identity for folding A_t*x into the eps psum
    iax = np.zeros((D, T_STEPS * D), np.float32)
    for i in range(T_STEPS):
        t = T_STEPS - 1 - i
        iax[:, i * D:(i + 1) * D] = (A[t] / (-C[t])) * np.eye(D, dtype=np.float32)

    iden16 = np.eye(D, dtype=np.float16)
    mfold = np.zeros((128, 32), np.float32)
    for k in range(4):
        mfold[32 * k + np.arange(32), np.arange(32)] = 1.0

    per_core = []
    for n in range(NCORES):
        # xc column order: [c0 | c2 | c1 | c3]
        rws = np.concatenate([rows[c, n] for pair in CHAIN_CHUNKS for c in pair])
        # noise': feature-major, (S_t*n - C_t*b2) * (-1/C_t), fp16
        nT = (step_noise[:, rws, :].transpose(0, 2, 1) * S[::-1, None, None]
              - (C[::-1, None] * b2[None, :])[:, :, None])
        nT = nT * (-1.0 / C[::-1, None, None])
        noiseT = np.ascontiguousarray(
            nT.transpose(1, 0, 2).reshape(D, T_STEPS * BL)).astype(np.float16)
        x0T = np.ascontiguousarray(init_noise[rws].T)
        rsq = np.zeros((128, 2), np.float32)
        for ci, c in enumerate(DMA_CHUNKS):
            rsq[:, ci] = rsq_all[rows[c, n]]
        rsqT2 = np.zeros((1, 512), np.float32)            # [r*2+e] per sbuf chunk
        for ci, c in enumerate(SBUF_CHUNKS):
            rsqT2[0, 256 * ci:256 * (ci + 1)] = np.repeat(rsq_all[rows[c, n]], 2)
        core = dict(tbl=tbl, tbl32=tbl32, noiseT=noiseT, x0T=x0T,
                    w1s=w1s, w2=np.ascontiguousarray(W2), wec=wec,
                    wec_p=np.ascontiguousarray(wec_p.reshape(2 * 32, D)),
                    bec=bec, tb1=tb1, iax=iax, iden16=iden16,
                    rsq=rsq, rsqT2=rsqT2, mfold=mfold)
        for ci in range(2):
            for k in range(NSEG):
                core[f"idx_{ci}_{k}"] = idx16[n][ci][k]
            core[f"aidx_{ci}"] = apgidx[n][ci]
        per_core.append((core, rws))

    consts = dict(A=A, C=C, S=S)
    return per_core, (G, GS), consts


def build_program(G, GS, consts):
    A, C, S = consts["A"], consts["C"], consts["S"]
    GSmax = int(GS.max())
    nc = bacc.Bacc("TRN2", target_bir_lowering=False, debug=False,
                   num_devices=NCORES)

    din = lambda name, shape, dt=F32: nc.dram_tensor(
        name, shape, dt, kind="ExternalInput").ap()
    tbl_d = din("tbl", [NSEG * SEGR, D])
    tbl32_d = din("tbl32", [128, Q + 1], U32)
    noiseT_d = din("noiseT", [D, T_STEPS * BL], F16)
    x0T_d = din("x0T", [D, BL], F32R)
    w1s_d = din("w1s", [128, 256], F32R)
    w2_d = din("w2", [256, D], F32R)
    wec_d = din("wec", [D, D])
    wec_p_d = din("wec_p", [64, D])
    bec_d = din("bec", [D, 1])
    tb1_d = din("tb1", [128, 2 * T_STEPS])
    iax_d = din("iax", [D, T_STEPS * D], F32R)
    iden16_d = din("iden16", [D, D], F16)
    rsq_d = din("rsq", [128, 2])
    rsqT2_d = din("rsqT2", [1, 512])
    mfold_d = din("mfold", [128, 32])
    idx_d, aidx_d = {}, {}
    for ci in range(2):
        for k in range(NSEG):
            idx_d[(ci, k)] = din(f"idx_{ci}_{k}", [128, 8 * int(G[ci, k])], I16)
        aidx_d[ci] = din(f"aidx_{ci}", [128, 8 * int(GS[ci])], I16)
    outT_d = nc.dram_tensor("outT", [D, BL], F32, kind="ExternalOutput").ap()

    Gmax = int(G.max())

    with tile.TileContext(nc) as tc:
        with (
            tc.tile_pool(name="const", bufs=1) as constp,
            tc.tile_pool(name="gidx", bufs=4) as gidxp,
            tc.tile_pool(name="aidx", bufs=2) as aidxp,
            tc.tile_pool(name="gdst", bufs=2) as gdstp,
            tc.tile_pool(name="apo", bufs=1) as apop,
            tc.tile_pool(name="smal", bufs=8) as smal,
            tc.tile_pool(name="redp", bufs=4) as redp,
            tc.tile_pool(name="accp", bufs=1) as accp,
            tc.tile_pool(name="xcp", bufs=1) as xcp,
            tc.tile_pool(name="hp", bufs=2) as hp,
            tc.tile_pool(name="ps_t", bufs=1, space="PSUM") as ps_t,
            tc.tile_pool(name="ps_h", bufs=2, space="PSUM") as ps_h,
            tc.tile_pool(name="ps_e", bufs=3, space="PSUM") as ps_e,
        ):
            # ---- DMA order: idx tiles, small consts, table, noise
            idx_t = {}
            for ci in range(2):
                for k in range(NSEG):
                    g = int(G[ci, k])
                    it = gidxp.tile([128, 8 * Gmax], I16, name=f"it{ci}{k}",
                                    tag="it")
                    nc.sync.dma_start(it[:, : 8 * g], idx_d[(ci, k)][:])
                    idx_t[(ci, k)] = it
            aidx_t = {}
            for ci in range(2):
                at = aidxp.tile([128, 8 * GSmax], I16, name=f"at{ci}", tag="at")
                nc.sync.dma_start(at[:, : 8 * int(GS[ci])], aidx_d[ci][:])
                aidx_t[ci] = at

            tbl32 = constp.tile([128, Q + 1], U32, name="tbl32")
            nc.sync.dma_start(tbl32[:], tbl32_d[:])
            ident = constp.tile([128, 128], F32, name="ident")
            make_identity(nc, ident[:])
            w1s = constp.tile([128, 256], F32R, name="w1s")
            nc.sync.dma_start(w1s[:], w1s_d[:])
            w2a = constp.tile([128, D], F32R, name="w2a")
            nc.sync.dma_start(w2a[:], w2_d[0:128, :])
            w2b = constp.tile([128, D], F32R, name="w2b")
            nc.sync.dma_start(w2b[:], w2_d[128:256, :])
            wec = constp.tile([D, D], F32, name="wec")
            nc.sync.dma_start(wec[:], wec_d[:])
            wec_p0 = constp.tile([32, D], F32, name="wec_p0")
            nc.sync.dma_start(wec_p0[:], wec_p_d[0:32, :])
            wec_p1 = constp.tile([32, D], F32, name="wec_p1")
            nc.sync.dma_start(wec_p1[:], wec_p_d[32:64, :])
            bec = constp.tile([D, 1], F32, name="bec")
            nc.sync.dma_start(bec[:], bec_d[:])
            tb1 = constp.tile([128, 2 * T_STEPS], F32, name="tb1")
            nc.sync.dma_start(tb1[:], tb1_d[:])
            iden16 = constp.tile([D, D], F16, name="iden16")
            nc.sync.dma_start(iden16[:], iden16_d[:])
            rsq = constp.tile([128, 2], F32, name="rsq")
            nc.sync.dma_start(rsq[:], rsq_d[:])
            rsqT2 = constp.tile([1, 512], F32, name="rsqT2")
            nc.sync.dma_start(rsqT2[:], rsqT2_d[:])
            iax = constp.tile([D, T_STEPS * D], F32R, name="iax")
            nc.sync.dma_start(iax[:], iax_d[:])
            # noise streamed in 256-col blocks b = 2*i+h, 10 blocks per tile
            NZT = 2560
            nz_tiles = {}

            def nz_fetch(q):
                if q not in nz_tiles:
                    t_ = xcp.tile([D, NZT], F16, name=f"nzq{q}", tag="nzq",
                                  bufs=2)
                    lo = q * NZT
                    hi = min(T_STEPS * BL, lo + NZT)
                    nc.sync.dma_start(t_[:, : hi - lo], noiseT_d[:, lo:hi])
                    nz_tiles[q] = t_
                return nz_tiles[q]

            xch = [xcp.tile([128, HB], F32R, name=f"xc{h}", tag=f"xc{h}")
                   for h in range(2)]
            for h in range(2):
                nc.sync.dma_start(xch[h][0:D, :],
                                  x0T_d[:, h * HB:(h + 1) * HB])

            # ones row for the rank-1 rsq broadcast; quarter-fold matrix
            ones128 = constp.tile([1, 128], F32, name="ones128")
            nc.vector.memset(ones128[:], 1.0)
            mfold = constp.tile([128, 32], F32, name="mfold")
            nc.sync.dma_start(mfold[:], mfold_d[:])

            # ---- DMA-path gathers (chunks 0,1 -> chains' first 128 cols)
            def do_gather(ci, k):
                g = int(G[ci, k])
                dst = gdstp.tile([128, Gmax * D], F32, name="dst", tag="dst")
                nc.gpsimd.dma_gather(
                    out_ap=dst[:, : g * D].rearrange("p (g d) -> p g d", g=g, d=D),
                    in_ap=tbl_d[k * SEGR:(k + 1) * SEGR, :],
                    idxs_ap=idx_t[(ci, k)][:, : 8 * g],
                    num_idxs=128 * g,
                    num_idxs_reg=128 * g,
                    elem_size=D,
                    single_packet=False,
                )
                return dst

            def do_reduce(ci, k, dst, acc):
                g = int(G[ci, k])
                w = g
                while w > 1:
                    m = w // 2
                    nc.vector.tensor_tensor(
                        out=dst[:, : m * D], in0=dst[:, : m * D],
                        in1=dst[:, (w - m) * D: w * D],
                        op=mybir.AluOpType.add)
                    w = w - m
                if acc is None:
                    acc2 = redp.tile([128, D], F32, name="rk", tag="rk")
                    nc.vector.tensor_copy(acc2[:], dst[:, :D])
                else:
                    acc2 = redp.tile([128, D], F32, name="acc2", tag="rk")
                    nc.vector.tensor_tensor(
                        out=acc2[:], in0=acc[:], in1=dst[:, :D],
                        op=mybir.AluOpType.add)
                return acc2

            def finish_dma_chunk(ci, acc):
                """scale by rsq, transpose, conditioning matmul, write c."""
                h = ci                                  # chain h uses DMA chunk ci
                ps = redp.tile([128, D], F32, name="ps", tag="rk")
                nc.vector.tensor_scalar(
                    out=ps[:], in0=acc[:], scalar1=rsq[:, ci:ci + 1],
                    scalar2=None, op0=mybir.AluOpType.mult)
                pt = ps_t.tile([D, 128], F32, name="pt", tag="pt")
                nc.tensor.transpose(out=pt[:], in_=ps[:], identity=ident[:])
                ptb = constp.tile([D, 128], F32, name=f"ptb{ci}")
                nc.scalar.copy(ptb[:], pt[:])
                pc = ps_t.tile([D, 128], F32, name="pc", tag="pc")
                nc.tensor.matmul(out=pc[:], lhsT=wec[:], rhs=ptb[:],
                                 start=True, stop=True)
                nc.scalar.activation(xch[h][D:128, 0:128], pc[:],
                                     mybir.ActivationFunctionType.Identity,
                                     bias=bec[:, 0:1])

            # ---- SBUF-path gathers (one ap_gather per chunk)
            def do_apg_chunk(ci):
                h = ci
                gs = int(GS[ci])
                n_idx = 128 * gs
                g = apop.tile([128, 128 * GSmax], U32, name="apg", tag="apg")
                nc.gpsimd.ap_gather(
                    out_ap=g[:, : n_idx],
                    in_ap=tbl32[:].rearrange("p (v d) -> p v d", v=Q + 1, d=1),
                    idxs_ap=aidx_t[ci][:, : 8 * gs],
                    channels=128, num_elems=Q + 1, d=1, num_idxs=n_idx)
                # pairwise tree over the gs slots of each row (packed fp16)
                gf = g[:, : n_idx].bitcast(F16)       # [128, (r gs*2)]
                w = gs
                while w > 1:
                    m = w // 2
                    nc.vector.tensor_tensor(
                        out=gf.rearrange("p (r x) -> p r x", r=128)[:, :, : 2 * m],
                        in0=gf.rearrange("p (r x) -> p r x", r=128)[:, :, : 2 * m],
                        in1=gf.rearrange("p (r x) -> p r x", r=128)[:, :, 2 * (w - m): 2 * w],
                        op=mybir.AluOpType.add)
                    w = w - m
                acc = accp.tile([128, 256], F32, name="apacc", tag="apacc")
                nc.vector.tensor_copy(
                    acc[:].rearrange("p (r x) -> p r x", r=128),
                    gf.rearrange("p (r x) -> p r x", r=128)[:, :, 0:2])
                # rank-1 rsq broadcast (PSUM) and scale acc in place
                rp = ps_t.tile([128, 256], F32, name="rp", tag="rp")
                nc.tensor.matmul(out=rp[:], lhsT=ones128[:],
                                 rhs=rsqT2[:, 256 * ci:256 * (ci + 1)],
                                 start=True, stop=True)
                acs = accp.tile([128, 256], F32, name="acs", tag="acs")
                nc.vector.tensor_tensor(out=acs[:], in0=acc[:],
                                        in1=rp[:], op=mybir.AluOpType.mult)
                # fold quarters via matmul: ff[q] = sum_k acs[32k+q]
                ff = ps_t.tile([32, 256], F32, name="ff", tag="rp")
                nc.tensor.matmul(out=ff[:], lhsT=mfold[:], rhs=acs[:],
                                 start=True, stop=True)
                sc = accp.tile([32, 256], F32, name="sc", tag="sc")
                nc.scalar.copy(sc[:], ff[:])
                # conditioning: pc[64,128] = sum_e wec_p[e].T @ sc[:, e::2]
                pc = ps_t.tile([D, 128], F32, name="pc2", tag="pc")
                nc.tensor.matmul(
                    out=pc[:], lhsT=wec_p0[:],
                    rhs=sc[:].rearrange("p (r e) -> p e r", e=2)[:, 0, :],
                    start=True, stop=False)
                nc.tensor.matmul(
                    out=pc[:], lhsT=wec_p1[:],
                    rhs=sc[:].rearrange("p (r e) -> p e r", e=2)[:, 1, :],
                    start=False, stop=True)
                nc.scalar.activation(xch[h][D:128, 128:256], pc[:],
                                     mybir.ActivationFunctionType.Identity,
                                     bias=bec[:, 0:1])

            def do_step(h, i):
                t = T_STEPS - 1 - i
                xc = xch[h]
                ph_a = ps_h.tile([128, HB], F32, name="ph_a", tag="ph")
                nc.tensor.matmul(out=ph_a[:], lhsT=w1s[:, 0:128],
                                 rhs=xc[:], start=True, stop=True)
                ph_b = ps_h.tile([128, HB], F32, name="ph_b", tag="ph")
                nc.tensor.matmul(out=ph_b[:], lhsT=w1s[:, 128:256],
                                 rhs=xc[:], start=True, stop=True)
                h_a = hp.tile([128, HB], F32R, name="h_a", tag="h")
                h_b = hp.tile([128, HB], F32R, name="h_b", tag="h")
                nc.scalar.activation(h_a[:], ph_a[:],
                                     mybir.ActivationFunctionType.Silu,
                                     bias=tb1[:, t:t + 1])
                nc.scalar.activation(h_b[:], ph_b[:],
                                     mybir.ActivationFunctionType.Silu,
                                     bias=tb1[:, T_STEPS + t:T_STEPS + t + 1])
                pe_t = ps_e.tile([D, HB], F32, name="pe_t", tag="pe")
                nc.tensor.matmul(out=pe_t[:],
                                 lhsT=iax[:, i * D:(i + 1) * D],
                                 rhs=xc[0:D, :], start=True, stop=False)
                col = i * BL + h * HB
                nzt = nz_fetch(col // 2560)
                co = col % 2560
                nc.tensor.matmul(out=pe_t[:], lhsT=iden16[:],
                                 rhs=nzt[:, co:co + HB],
                                 start=False, stop=False)
                nc.tensor.matmul(out=pe_t[:], lhsT=w2a[:],
                                 rhs=h_a[:], start=False, stop=False)
                nc.tensor.matmul(out=pe_t[:], lhsT=w2b[:],
                                 rhs=h_b[:], start=False, stop=True)
                nc.scalar.activation(
                    xc[0:D, :], pe_t[:],
                    mybir.ActivationFunctionType.Identity, scale=-float(C[t]))

            # ---- schedule: Pool [dg-c0 x4, dg-c1 x4, apg2, apg3];
            # DVE: [c0 trees, apg2 tree, c1 trees, apg3 tree]
            dsts0 = [do_gather(0, k) for k in range(NSEG)]
            dsts1 = [do_gather(1, k) for k in range(NSEG)]
            acc = None
            for k in range(NSEG):
                acc = do_reduce(0, k, dsts0[k], acc)
            finish_dma_chunk(0, acc)
            do_apg_chunk(0)
            acc = None
            for k in range(NSEG):
                acc = do_reduce(1, k, dsts1[k], acc)
            finish_dma_chunk(1, acc)
            do_apg_chunk(1)

            ia, ib = 0, 0
            while ia < T_STEPS or ib < T_STEPS:
                if ia < T_STEPS:
                    do_step(0, ia)
                    ia += 1
                if ib < T_STEPS:
                    do_step(1, ib)
                    ib += 1

            for h in range(2):
                nc.sync.dma_start(outT_d[:, h * HB:(h + 1) * HB],
                                  xch[h][0:D, :].bitcast(F32))

    nc.compile()
    return nc


_CACHE = {}


def _get_program(GG, consts):
    G, GS = GG
    key = tuple(G.reshape(-1).tolist()) + tuple(GS.reshape(-1).tolist())
    if key not in _CACHE:
        _CACHE[key] = build_program(G, GS, consts)
    return _CACHE[key]


def kernel(**inputs):
    per_core, G, consts = host_prep(inputs)
    nc = _get_program(G, consts)
    in_maps = [core for core, _ in per_core]
    res = run_bass_kernel_spmd(nc, in_maps, list(range(NCORES)))
    out = np.zeros((B, D), np.float32)
    for n in range(NCORES):
        _, rws = per_core[n]
        out[rws] = res.results[n]["outT"].T
    return out
